# revision 1
# baseline (speedup 1.0000x reference)
"""Causal self-attention kernel for 8 trn2 NeuronCores.

Sharding: core c = (b, g) with b = c // 2 (batch), g = c % 2 (head-group of 8
heads, Megatron column split of Wq/Wk/Wv + row split of Wp). Each core computes
a partial Y for its batch; host sums the two partials per batch.

Per-core dataflow (all matmuls in float32r = full-rate TF32-ish):
  - host pre-transposes x[b] -> xT [C, T] so contraction dims land on
    partitions without any on-device transposes.
  - QKV projections produce Q^T, K^T in [head-pair (128), T] layout and V in
    [T, headcols] layout (with a ones-column per head for free softmax sums).
  - scores are computed transposed: S^T = K @ Q^T per (pair, ktile) with
    row-tiled head pairs (K=64 contraction x2 concurrent).
  - softmax: exp on ACT straight out of PSUM (scale=1/8 folded in), causal
    mask via gpsimd affine_select on the exp output, colsum rides as row 64
    of the PV matmul (ones-augmented V, M=65).
  - PV: O^T[h] = V_aug[h]^T @ P^T accumulated over ktiles in PSUM.
  - normalization: recip(colsum) broadcast down 64 partitions with a K=1
    ones matmul, multiplied during PSUM->SBUF eviction.
  - out-proj: Y += A^T-tiles.T @ Wp rows, row-tiled head pairs, + bias.
"""

import numpy as np

import concourse.bacc as bacc
import concourse.bass as bass
import concourse.mybir as mybir
import concourse.tile as tile

F32 = mybir.dt.float32
F32R = mybir.dt.float32r
AF = mybir.ActivationFunctionType
ALU = mybir.AluOpType

import os as _os
B, T, C = 4, int(_os.environ.get("KT_T", "2048")), 1024
H, D = 16, 64
G = 2  # head-group shards (cores per batch)
GC = C // G  # 512 output cols per shard
P = 128
NCT = C // P  # 8 contraction tiles over C
TCH = 512  # T chunk (= PSUM bank in fp32)
NTCH = T // TCH  # 4
NPAIR = GC // P  # 4 head pairs per core
NKT = T // P  # 16 key tiles
HPC = H // G  # 8 heads per core
VW = D + 1  # V columns per head incl. ones column


def bcast_ap(h, parts, free):
    """DRAM [free] vector -> [parts, free] partition-broadcast AP."""
    ap = h[:]
    return bass.AP(tensor=ap.tensor, offset=ap.offset, ap=[[0, parts], [1, free]])


def build_nc(stop_phase="C"):
    nc = bacc.Bacc("TRN2", target_bir_lowering=False)

    xT = nc.dram_tensor("xT", [C, T], F32R, kind="ExternalInput")
    wq = nc.dram_tensor("wq", [C, GC], F32R, kind="ExternalInput")
    wk = nc.dram_tensor("wk", [C, GC], F32R, kind="ExternalInput")
    wv = nc.dram_tensor("wv", [C, GC], F32R, kind="ExternalInput")
    wp = nc.dram_tensor("wp", [GC, C], F32R, kind="ExternalInput")
    bqd = nc.dram_tensor("bq", [GC], F32, kind="ExternalInput")
    bkd = nc.dram_tensor("bk", [GC], F32, kind="ExternalInput")
    bvd = nc.dram_tensor("bv", [GC], F32, kind="ExternalInput")
    bpd = nc.dram_tensor("bp", [C], F32, kind="ExternalInput")
    ones64 = nc.dram_tensor("ones64", [P, P], F32R, kind="ExternalInput")
    y = nc.dram_tensor(
        "y",
        [T, C],
        F32R if stop_phase in ("A", "B", "CR", "CD5") else F32,
        kind="ExternalOutput",
    )

    xT_r = xT[:, :].rearrange("(o p) t -> p o t", p=P)  # [128, 8, T]
    wq_r = wq[:, :].rearrange("(o p) m -> p o m", p=P)  # [128, 8, 512]
    wk_r = wk[:, :].rearrange("(o p) m -> p o m", p=P)
    wv_r = wv[:, :].rearrange("(o p) m -> p o m", p=P)
    wp_r = wp[:, :].rearrange("(o p) m -> p o m", p=P)  # [128, 4, 1024]
    y_r = y[:, :].rearrange("(n p) c -> n p c", p=P)  # [16, 128, 1024]

    with tile.TileContext(nc) as tc:
        with (
            tc.tile_pool(name="persist", bufs=1) as persist,
            tc.tile_pool(name="small", bufs=1) as small,
        ):
            # ---- persistent SBUF state ----
            qT_sb = persist.tile([P, NPAIR, T], F32R)  # 32KB/part
            kT_sb = persist.tile([P, NPAIR, T], F32R)  # 32KB/part
            v_sb = persist.tile([P, NKT, HPC, VW], F32R)  # ~33KB/part
            wp_sb = persist.tile([P, NPAIR, C], F32R)  # 16KB/part
            nc.sync.dma_start(out=wp_sb, in_=wp_r)

            bq_sb = small.tile([P, NPAIR], F32)
            nc.sync.dma_start(out=bq_sb, in_=bqd[:].rearrange("(o p) -> p o", p=P))
            bk_sb = small.tile([P, NPAIR], F32)
            nc.sync.dma_start(out=bk_sb, in_=bkd[:].rearrange("(o p) -> p o", p=P))
            bv_sb = small.tile([P, GC], F32)
            nc.gpsimd.dma_start(out=bv_sb, in_=bcast_ap(bvd, P, GC))
            bp_sb = small.tile([P, C], F32)
            nc.gpsimd.dma_start(out=bp_sb, in_=bcast_ap(bpd, P, C))
            ones_sb = small.tile([P, P], F32R)
            nc.sync.dma_start(out=ones_sb, in_=ones64[:, :])
            # ones column per head, copied from the ones tile
            nc.vector.tensor_copy(
                v_sb[:, :, :, D : D + 1],
                ones_sb[:, 0 : NKT * HPC].rearrange(
                    "p (a b o) -> p a b o", b=HPC, o=1
                ),
            )

            # ================= Phase A: QKV projections =================
            with (
                tc.tile_pool(name="wpool", bufs=1) as wpool,
                tc.tile_pool(name="xpool", bufs=2) as xpool,
                tc.tile_pool(name="prpsum", bufs=2, space="PSUM") as prpsum,
            ):
                wq_sb = wpool.tile([P, NCT, GC], F32R)
                nc.sync.dma_start(out=wq_sb, in_=wq_r)
                wk_sb = wpool.tile([P, NCT, GC], F32R)
                nc.sync.dma_start(out=wk_sb, in_=wk_r)
                wv_sb = wpool.tile([P, NCT, GC], F32R)
                nc.sync.dma_start(out=wv_sb, in_=wv_r)

                for tch in range(NTCH):
                    tsl = slice(tch * TCH, (tch + 1) * TCH)
                    xt = xpool.tile([P, NCT, TCH], F32R, name="xt")
                    nc.sync.dma_start(out=xt, in_=xT_r[:, :, tsl])
                    for pair in range(NPAIR):
                        psl = slice(pair * P, (pair + 1) * P)
                        q_ps = prpsum.tile([P, TCH], F32, name="q_ps")
                        for ct in range(NCT):
                            nc.tensor.matmul(
                                q_ps,
                                lhsT=wq_sb[:, ct, psl],
                                rhs=xt[:, ct, :],
                                start=(ct == 0),
                                stop=(ct == NCT - 1),
                            )
                        nc.vector.tensor_scalar_add(
                            qT_sb[:, pair, tsl], q_ps, bq_sb[:, pair : pair + 1]
                        )
                        k_ps = prpsum.tile([P, TCH], F32, name="k_ps")
                        for ct in range(NCT):
                            nc.tensor.matmul(
                                k_ps,
                                lhsT=wk_sb[:, ct, psl],
                                rhs=xt[:, ct, :],
                                start=(ct == 0),
                                stop=(ct == NCT - 1),
                            )
                        nc.vector.tensor_scalar_add(
                            kT_sb[:, pair, tsl], k_ps, bk_sb[:, pair : pair + 1]
                        )
                    for tloc in range(4):
                        tt = tch * 4 + tloc
                        v_ps = prpsum.tile([P, GC], F32, name="v_ps")
                        for ct in range(NCT):
                            nc.tensor.matmul(
                                v_ps,
                                lhsT=xt[:, ct, tloc * P : (tloc + 1) * P],
                                rhs=wv_sb[:, ct, :],
                                start=(ct == 0),
                                stop=(ct == NCT - 1),
                            )
                        nc.vector.tensor_tensor(
                            v_sb[:, tt, :, 0:D],
                            v_ps.rearrange("p (h d) -> p h d", d=D),
                            bv_sb.rearrange("p (h d) -> p h d", d=D),
                            ALU.add,
                        )

            if stop_phase == "A":
                for pair in range(NPAIR):
                    nc.sync.dma_start(out=y_r[pair, :, :], in_=qT_sb[:, pair, 0:1024])
                    nc.sync.dma_start(
                        out=y_r[4 + pair, :, :], in_=kT_sb[:, pair, 0:1024]
                    )
                for tt in range(4):
                    nc.sync.dma_start(
                        out=y_r[8 + tt, :, 0:GC].rearrange("p (h d) -> p h d", d=D),
                        in_=v_sb[:, tt, :, 0:D],
                    )

            # ================= Phase B + C: attention + out-proj =========
            if stop_phase != "A":
              with (
                  tc.tile_pool(name="stpsum", bufs=1, space="PSUM") as stpsum,
                  tc.tile_pool(name="opsum", bufs=1, space="PSUM") as opsum,
                  tc.tile_pool(name="bcpsum", bufs=1, space="PSUM") as bcpsum,
                  tc.tile_pool(name="ypsum", bufs=1, space="PSUM") as ypsum,
                  tc.tile_pool(name="ptpool", bufs=2) as ptpool,
                  tc.tile_pool(name="atpool", bufs=6) as atpool,
                  tc.tile_pool(name="mpool", bufs=2) as mpool,
                  tc.tile_pool(name="ypool", bufs=3) as ypool,
              ):
                  for qc in range(NTCH):
                      qsl = slice(qc * TCH, (qc + 1) * TCH)
                      nkt = (qc + 1) * 4
                      aTs = []
                      for pair in range(NPAIR):
                          o_ps_e = opsum.tile([D + 1, TCH], F32, name="o_ps_e")
                          o_ps_o = opsum.tile([D + 1, TCH], F32, name="o_ps_o")
                          gw = 1 if stop_phase == "C2" else 2
                          for g0 in range(0, nkt, gw):
                              st = stpsum.tile([P, gw, 2, TCH], F32, name="st")
                              for ti in range(gw):
                                  t = g0 + ti
                                  ksl = slice(t * P, (t + 1) * P)
                                  nc.tensor.matmul(
                                      st[:, ti, 0, :],
                                      lhsT=kT_sb[0:D, pair, ksl],
                                      rhs=qT_sb[0:D, pair, qsl],
                                      start=True,
                                      stop=True,
                                  )
                                  nc.tensor.matmul(
                                      st[:, ti, 1, :],
                                      lhsT=kT_sb[D:P, pair, ksl],
                                      rhs=qT_sb[D:P, pair, qsl],
                                      start=True,
                                      stop=True,
                                  )
                              pt = ptpool.tile([P, gw, 2, TCH], F32R, name="pt")
                              nc.scalar.activation(
                                  out=pt.rearrange("p a b n -> p (a b n)"),
                                  in_=st.rearrange("p a b n -> p (a b n)"),
                                  func=AF.Exp,
                                  scale=0.125,
                              )
                              for ti in range(gw):
                                  t = g0 + ti
                                  if t >= 4 * qc:  # diagonal block: causal mask
                                      for h in range(2):
                                          mv = pt[:, ti, h, :]
                                          nc.gpsimd.affine_select(
                                              out=mv,
                                              in_=mv,
                                              pattern=[[1, TCH]],
                                              compare_op=ALU.is_ge,
                                              fill=0.0,
                                              base=TCH * qc - P * t,
                                              channel_multiplier=-1,
                                          )
                              for ti in range(gw):
                                  t = g0 + ti
                                  nc.tensor.matmul(
                                      o_ps_e,
                                      lhsT=v_sb[:, t, 2 * pair, :],
                                      rhs=pt[:, ti, 0, :],
                                      start=(t == 0),
                                      stop=(t == nkt - 1),
                                      skip_group_check=True,
                                  )
                                  nc.tensor.matmul(
                                      o_ps_o,
                                      lhsT=v_sb[:, t, 2 * pair + 1, :],
                                      rhs=pt[:, ti, 1, :],
                                      start=(t == 0),
                                      stop=(t == nkt - 1),
                                      skip_group_check=True,
                                  )
                          # ---- normalize: aT = O^T * bcast(1/colsum) ----
                          recip = mpool.tile([P, 2, TCH], F32R, name="recip")
                          with nc.allow_low_precision(reason="softmax recip to f32r"):
                              nc.vector.reciprocal(
                                  recip[D : D + 1, 0, :], o_ps_e[D : D + 1, :]
                              )
                              nc.vector.reciprocal(
                                  recip[D : D + 1, 1, :], o_ps_o[D : D + 1, :]
                              )
                          aT = atpool.tile([P, TCH], F32R, name="aT")
                          bc_e = bcpsum.tile([D, TCH], F32, name="bc")
                          nc.tensor.matmul(
                              bc_e,
                              lhsT=ones_sb[D : D + 1, 0:D],
                              rhs=recip[D : D + 1, 0, :],
                              start=True,
                              stop=True,
                          )
                          bc_e_sb = mpool.tile([D, TCH], F32, name="bc_e_sb")
                          nc.vector.tensor_copy(bc_e_sb, bc_e)
                          nc.vector.tensor_tensor(
                              aT[0:D, :], o_ps_e[0:D, :], bc_e_sb, ALU.mult
                          )
                          bc_o = bcpsum.tile([D, TCH], F32, name="bc")
                          nc.tensor.matmul(
                              bc_o,
                              lhsT=ones_sb[D : D + 1, 0:D],
                              rhs=recip[D : D + 1, 1, :],
                              start=True,
                              stop=True,
                          )
                          bc_o_sb = mpool.tile([D, TCH], F32, name="bc_o_sb")
                          nc.vector.tensor_copy(bc_o_sb, bc_o)
                          stage = mpool.tile([D, TCH], F32R, name="stage")
                          nc.vector.tensor_tensor(stage, o_ps_o[0:D, :], bc_o_sb, ALU.mult)
                          nc.sync.dma_start(out=aT[D:P, :], in_=stage)
                          aTs.append(aT)
                          if stop_phase == "B":
                              nc.sync.dma_start(
                                  out=y_r[qc * 4 + pair, :, 0:TCH], in_=aT
                              )

                      if stop_phase == "B":
                          continue
                      # ---- out-proj for this q-chunk ----
                      for tloc in range(4):
                          trow = qc * 4 + tloc
                          lsl = slice(tloc * P, (tloc + 1) * P)
                          for cch in range(2):
                              csl = slice(cch * TCH, (cch + 1) * TCH)
                              y_ps = ypsum.tile([P, TCH], F32, name="y_ps")
                              for pair in range(NPAIR):
                                  nc.tensor.matmul(
                                      y_ps,
                                      lhsT=aTs[pair][:, lsl],
                                      rhs=wp_sb[:, pair, csl],
                                      start=(pair == 0),
                                      stop=(pair == NPAIR - 1),
                                      skip_group_check=True,
                                  )
                              y_sb = ypool.tile([P, TCH], F32, name="y_sb")
                              nc.vector.tensor_tensor(
                                  y_sb, y_ps, bp_sb[:, csl], ALU.add
                              )
                              nc.sync.dma_start(out=y_r[trow, :, csl], in_=y_sb)
    nc.finalize()
    return nc


_CACHE = {}


def _get_nc(stop_phase="C"):
    if stop_phase not in _CACHE:
        _CACHE[stop_phase] = build_nc(stop_phase)
    return _CACHE[stop_phase]


def make_in_maps(x, Wq, bq, Wk, bk, Wv, bv, Wp, bp):
    f = np.float32
    x = np.asarray(x, f)
    in_maps = []
    for core in range(8):
        b, g = core // 2, core % 2
        sl = slice(g * GC, (g + 1) * GC)
        in_maps.append(
            {
                "xT": np.ascontiguousarray(x[b].T),
                "wq": np.ascontiguousarray(np.asarray(Wq, f)[:, sl]),
                "wk": np.ascontiguousarray(np.asarray(Wk, f)[:, sl]),
                "wv": np.ascontiguousarray(np.asarray(Wv, f)[:, sl]),
                "wp": np.ascontiguousarray(np.asarray(Wp, f)[sl, :]),
                "bq": np.ascontiguousarray(np.asarray(bq, f)[sl]),
                "bk": np.ascontiguousarray(np.asarray(bk, f)[sl]),
                "bv": np.ascontiguousarray(np.asarray(bv, f)[sl]),
                "bp": np.asarray(bp, f) if g == 0 else np.zeros(C, f),
                "ones64": np.ones((P, P), f),
            }
        )
    return in_maps


def run(in_maps, stop_phase="C", **kwargs):
    from concourse.bass_utils import run_bass_kernel_spmd

    return run_bass_kernel_spmd(
        _get_nc(stop_phase), in_maps, core_ids=list(range(8)), **kwargs
    )


def kernel(x, Wq, bq, Wk, bk, Wv, bv, Wp, bp):
    in_maps = make_in_maps(x, Wq, bq, Wk, bk, Wv, bv, Wp, bp)
    res = run(in_maps)
    ys = [r["y"] for r in res.results]
    out = np.stack([ys[2 * b] + ys[2 * b + 1] for b in range(B)])
    return out



# revision 2
# speedup vs baseline: 1.0030x; 1.0030x over previous
"""Causal self-attention kernel for 8 trn2 NeuronCores — v3.

Sharding: core c = (b, g): b = c // 2 (batch), g = c % 2 (head-group of 8
heads, Megatron column split of Wq/Wk/Wv + row split of Wp). Host sums the
two partial Y per batch and adds bp.

Design (all matmul operands bf16, PSUM f32):
  - One streamed schedule: projection chunk j+1 and the hoisted first pair of
    the last attention chunk are interleaved into attention chunk j at
    matmul-group granularity, keeping the PE busy while ACT works through
    the softmax exps.
  - QK per k-tile: two concurrent 64x128 row-tile matmuls (head-even
    partitions 0:63 / head-odd 64:127) into separate PSUM banks.
  - V tiles augmented per head: even heads [V | 1], odd heads [1 | V]; the
    PV matmul leaves O on one partition half and the softmax colsum
    (replicated x64) on the other. Normalization: 2 cross-half copies
    assemble both colsum blocks into one full tile, one
    reciprocal_approx_fast, 2 DVE mults write A^T directly (no DMA).
  - Diagonal-block trimming: QK/exp/PV restricted to live columns
    [128*tloc:512]; dead pt columns zeroed by gpsimd memset; causal
    affine_select only on the 128-column boundary window.
  - Startup weight DMAs split across engine queues so the first projection
    matmul issues ~10us after kernel start.
"""

import numpy as np
import ml_dtypes

import concourse.bacc as bacc
import concourse.bass as bass
import concourse.mybir as mybir
import concourse.tile as tile

F32 = mybir.dt.float32
BF16 = mybir.dt.bfloat16
AF = mybir.ActivationFunctionType
ALU = mybir.AluOpType

B, T, C = 4, 2048, 1024
H, D = 16, 64
G = 2
GC = C // G  # 512
P = 128
NCT = C // P  # 8
TCH = 512
NTCH = T // TCH  # 4
NPAIR = GC // P  # 4
NKT = T // P  # 16
HPC = H // G  # 8
LAG = 2


def bcast_ap(h, parts, free):
    ap = h[:]
    return bass.AP(tensor=ap.tensor, offset=ap.offset, ap=[[0, parts], [1, free]])


def build_nc():
    nc = bacc.Bacc("TRN2", target_bir_lowering=False)

    # all inputs host-prearranged into on-chip layout: DMA lines are long
    # and contiguous per partition (8KB), not 1KB gather packets.
    xh = nc.dram_tensor("xh", [NTCH, P, NCT, TCH], BF16, kind="ExternalInput")
    wq = nc.dram_tensor("wq", [P, NCT, GC], BF16, kind="ExternalInput")
    wk = nc.dram_tensor("wk", [P, NCT, GC], BF16, kind="ExternalInput")
    wv = nc.dram_tensor("wv", [P, NCT, GC], BF16, kind="ExternalInput")
    wp = nc.dram_tensor("wp", [P, NPAIR, C], BF16, kind="ExternalInput")
    bqd = nc.dram_tensor("bq", [P, NPAIR], F32, kind="ExternalInput")
    bkd = nc.dram_tensor("bk", [P, NPAIR], F32, kind="ExternalInput")
    bvd = nc.dram_tensor("bv", [GC], F32, kind="ExternalInput")
    y = nc.dram_tensor("y", [T, C], F32, kind="ExternalOutput")

    wq_r, wk_r, wv_r, wp_r = wq[:, :, :], wk[:, :, :], wv[:, :, :], wp[:, :, :]
    y_r = y[:, :].rearrange("(n p) c -> n p c", p=P)

    with tile.TileContext(nc) as tc:
        with (
            tc.tile_pool(name="persist", bufs=1) as persist,
            tc.tile_pool(name="small", bufs=1) as small,
            tc.tile_pool(name="qpool", bufs=2) as qpool,
            tc.tile_pool(name="xpool", bufs=2) as xpool,
            tc.tile_pool(name="ptpool", bufs=4) as ptpool,
            tc.tile_pool(name="hoistpool", bufs=8) as hoistpool,
            tc.tile_pool(name="atpool", bufs=14) as atpool,
            tc.tile_pool(name="cspool", bufs=2) as cspool,
            tc.tile_pool(name="rcpool", bufs=2) as rcpool,
            tc.tile_pool(name="ypool", bufs=3) as ypool,
            tc.tile_pool(name="prpsum", bufs=2, space="PSUM") as prpsum,
            tc.tile_pool(name="stpsum", bufs=2, space="PSUM") as stpsum,
            tc.tile_pool(name="opsum", bufs=2, space="PSUM") as opsum,
        ):
            kT_sb = persist.tile([P, NPAIR, T], BF16)
            v_sb = persist.tile([P, NKT, HPC, P], BF16)
            wp_sb = persist.tile([P, NPAIR, C], BF16)
            wq_sb = persist.tile([P, NCT, GC], BF16)
            wk_sb = persist.tile([P, NCT, GC], BF16)
            wv_sb = persist.tile([P, NCT, GC], BF16)
            # spread the startup loads over engine queues; wq + x chunk 0
            # gate the first matmul, so they go first on separate queues.
            nc.sync.dma_start(out=wq_sb[:, 0:4, :], in_=wq_r[:, 0:4, :])
            nc.scalar.dma_start(out=wq_sb[:, 4:8, :], in_=wq_r[:, 4:8, :])
            xt0 = xpool.tile([P, NCT, TCH], BF16, name="xt")
            nc.sync.dma_start(out=xt0[:, 0:4, :], in_=xh[0, :, 0:4, :])
            nc.scalar.dma_start(out=xt0[:, 4:6, :], in_=xh[0, :, 4:6, :])
            nc.gpsimd.dma_start(out=xt0[:, 6:8, :], in_=xh[0, :, 6:8, :])
            # PE warmup during the input load: junk matmuls keep the HAM
            # clock gate at full rate so the first real matmuls run at 2.4GHz
            wtile = small.tile([P, TCH], BF16)
            nc.gpsimd.memset(wtile, 1.0)
            warm = opsum.tile([P, TCH], F32, name="ops")
            for wi in range(40):
                nc.tensor.matmul(
                    warm, lhsT=wtile[:, 0:P], rhs=wtile,
                    start=(wi == 0), stop=(wi == 39), skip_group_check=True,
                )

            bq_sb = small.tile([P, NPAIR], F32)
            nc.scalar.dma_start(out=bq_sb, in_=bqd[:, :])
            bk_sb = small.tile([P, NPAIR], F32)
            nc.scalar.dma_start(out=bk_sb, in_=bkd[:, :])

            # gpsimd queue: bv broadcast now; x chunk 0 next (inside gen_A(0));
            # ones-block memsets and wp (first needed ~25us / ~60us in) after.
            bv_sb = small.tile([P, GC], F32)
            nc.gpsimd.dma_start(out=bv_sb, in_=bcast_ap(bvd, P, GC))

            qts = {}
            aTs = {}
            pre_pts = {}

            def gen_A(tch):
                """QKV projections for t-chunk tch; yields after each MM group."""
                tsl = slice(tch * TCH, (tch + 1) * TCH)  # kT column range
                if tch == 0:
                    xt = xt0
                else:
                    xt = xpool.tile([P, NCT, TCH], BF16, name="xt")
                    nc.sync.dma_start(out=xt, in_=xh[tch, :, :, :])
                qT = qpool.tile([P, NPAIR, TCH], BF16, name="qT")
                qts[tch] = qT
                for pair in range(NPAIR):
                    psl = slice(pair * P, (pair + 1) * P)
                    q_ps = prpsum.tile([P, TCH], F32, name="pr")
                    for ct in range(NCT):
                        nc.tensor.matmul(
                            q_ps, lhsT=wq_sb[:, ct, psl], rhs=xt[:, ct, :],
                            start=(ct == 0), stop=(ct == NCT - 1),
                            skip_group_check=True,
                        )
                        if ct == 3:
                            yield
                    nc.vector.tensor_scalar_add(
                        qT[:, pair, :], q_ps, bq_sb[:, pair : pair + 1]
                    )
                    yield
                for pair in range(NPAIR):
                    psl = slice(pair * P, (pair + 1) * P)
                    k_ps = prpsum.tile([P, TCH], F32, name="pr")
                    for ct in range(NCT):
                        nc.tensor.matmul(
                            k_ps, lhsT=wk_sb[:, ct, psl], rhs=xt[:, ct, :],
                            start=(ct == 0), stop=(ct == NCT - 1),
                            skip_group_check=True,
                        )
                        if ct == 3:
                            yield
                    nc.vector.tensor_scalar_add(
                        kT_sb[:, pair, tsl], k_ps, bk_sb[:, pair : pair + 1]
                    )
                    yield
                for tloc in range(4):
                    tt = tch * 4 + tloc
                    v_ps = prpsum.tile([P, GC], F32, name="pr")
                    for ct in range(NCT):
                        nc.tensor.matmul(
                            v_ps, lhsT=xt[:, ct, tloc * P : (tloc + 1) * P],
                            rhs=wv_sb[:, ct, :],
                            start=(ct == 0), stop=(ct == NCT - 1),
                            skip_group_check=True,
                        )
                        if ct == 3:
                            yield
                    vp4 = v_ps.rearrange("p (h2 two d) -> p h2 two d", two=2, d=D)
                    bv4 = bv_sb.rearrange("p (h2 two d) -> p h2 two d", two=2, d=D)
                    vd4 = v_sb[:, tt, :, :].rearrange(
                        "p (h2 two) c -> p h2 two c", two=2
                    )
                    nc.vector.tensor_tensor(
                        vd4[:, :, 0, 0:D], vp4[:, :, 0, :], bv4[:, :, 0, :], ALU.add
                    )
                    nc.vector.tensor_tensor(
                        vd4[:, :, 1, D:P], vp4[:, :, 1, :], bv4[:, :, 1, :], ALU.add
                    )
                    yield

            def gen_QKhoist(qc, pair, t_list):
                """QK + exp only (non-diagonal tiles), pt kept in hoistpool."""
                qT = qts[qc]
                for t in t_list:
                    ksl = slice(t * P, (t + 1) * P)
                    st = stpsum.tile([P, 2, TCH], F32, name="st")
                    nc.tensor.matmul(
                        st[:, 0, :], lhsT=kT_sb[0:D, pair, ksl],
                        rhs=qT[0:D, pair, :], start=True, stop=True,
                    )
                    nc.tensor.matmul(
                        st[:, 1, :], lhsT=kT_sb[D:P, pair, ksl],
                        rhs=qT[D:P, pair, :], start=True, stop=True,
                    )
                    pt = hoistpool.tile([P, 2, TCH], BF16, name="hpt")
                    pre_pts[(qc, pair, t)] = pt
                    nc.scalar.activation(
                        out=pt[:, :, :], in_=st[:, :, :], func=AF.Exp, scale=0.125,
                    )
                    yield

            def gen_Bpair(qc, pair, pre_n=0):
                """Attention for (q-chunk qc, head pair); yields per step."""
                nkt = (qc + 1) * 4
                qT = qts[qc]
                o_e = opsum.tile([P, TCH], F32, name="ops")
                o_o = opsum.tile([P, TCH], F32, name="ops")
                pts = {t: pre_pts[(qc, pair, t)] for t in range(pre_n)}
                c0s = {t: 0 for t in range(pre_n)}
                # 2 ktiles per iteration: QK,QK then PV,PV halves the
                # row-mode <-> full-mode transitions on the PE.
                for it in range(nkt // 2 + 1):
                    for t in (2 * it, 2 * it + 1):
                        if t >= nkt or t < pre_n:
                            continue
                        ksl = slice(t * P, (t + 1) * P)
                        c0 = P * (t - 4 * qc) if t >= 4 * qc else 0
                        c0s[t] = c0
                        csl = slice(c0, TCH)
                        st = stpsum.tile([P, 2, TCH], F32, name="st")
                        nc.tensor.matmul(
                            st[:, 0, csl], lhsT=kT_sb[0:D, pair, ksl],
                            rhs=qT[0:D, pair, csl], start=True, stop=True,
                        )
                        nc.tensor.matmul(
                            st[:, 1, csl], lhsT=kT_sb[D:P, pair, ksl],
                            rhs=qT[D:P, pair, csl], start=True, stop=True,
                        )
                        pt = ptpool.tile([P, 2, TCH], BF16, name="pt")
                        pts[t] = pt
                        if c0 > 0:
                            nc.gpsimd.memset(pt[:, :, 0:c0], 0.0)
                        nc.scalar.activation(
                            out=pt[:, :, csl], in_=st[:, :, csl],
                            func=AF.Exp, scale=0.125,
                        )
                        if t >= 4 * qc:
                            # boundary window [c0:c0+P): keep iff w >= p
                            nc.gpsimd.affine_select(
                                out=pt[:, :, c0 : c0 + P],
                                in_=pt[:, :, c0 : c0 + P],
                                pattern=[[0, 2], [1, P]],
                                compare_op=ALU.is_ge, fill=0.0,
                                base=0, channel_multiplier=-1,
                            )
                    for pv in (2 * it - LAG, 2 * it - LAG + 1):
                        if not (0 <= pv < nkt):
                            continue
                        pcsl = slice(c0s[pv], TCH)
                        nc.tensor.matmul(
                            o_e[:, pcsl], lhsT=v_sb[:, pv, 2 * pair, :],
                            rhs=pts[pv][:, 0, pcsl],
                            start=(pv == 0), stop=(pv == nkt - 1),
                            skip_group_check=True,
                        )
                        nc.tensor.matmul(
                            o_o[:, pcsl], lhsT=v_sb[:, pv, 2 * pair + 1, :],
                            rhs=pts[pv][:, 1, pcsl],
                            start=(pv == 0), stop=(pv == nkt - 1),
                            skip_group_check=True,
                        )
                        del pts[pv]
                    yield
                # normalization: colsums at o_e[D:P] and o_o[0:D]
                cs = cspool.tile([P, TCH], F32, name="cs")
                nc.vector.tensor_copy(cs[0:D, :], o_e[D:P, :])
                nc.vector.tensor_copy(cs[D:P, :], o_o[0:D, :])
                rc = rcpool.tile([P, TCH], F32, name="rc")
                nc.vector.reciprocal_approx_fast(out=rc, in_=cs)
                aT = atpool.tile([P, TCH], BF16, name="aT")
                nc.vector.tensor_tensor(aT[0:D, :], o_e[0:D, :], rc[0:D, :], ALU.mult)
                nc.vector.tensor_tensor(aT[D:P, :], o_o[D:P, :], rc[D:P, :], ALU.mult)
                aTs[(qc, pair)] = aT
                yield

            def gen_proj(qc, evict_on_act=False):
                for tloc in range(4):
                    trow = qc * 4 + tloc
                    lsl = slice(tloc * P, (tloc + 1) * P)
                    for cch in range(2):
                        csl = slice(cch * TCH, (cch + 1) * TCH)
                        # pr rotation: free of A-work in the last stage, and
                        # never entangled with the o_e/o_o rotation.
                        y_ps = prpsum.tile([P, TCH], F32, name="pr")
                        for pair in range(NPAIR):
                            nc.tensor.matmul(
                                y_ps, lhsT=aTs[(qc, pair)][:, lsl],
                                rhs=wp_sb[:, pair, csl],
                                start=(pair == 0), stop=(pair == NPAIR - 1),
                                skip_group_check=True,
                            )
                        y_sb = ypool.tile([P, TCH], F32, name="y_sb")
                        if evict_on_act:
                            nc.scalar.copy(y_sb, y_ps)
                            nc.scalar.dma_start(out=y_r[trow, :, csl], in_=y_sb)
                        else:
                            nc.vector.tensor_copy(y_sb, y_ps)
                            nc.sync.dma_start(out=y_r[trow, :, csl], in_=y_sb)
                        yield

            def chain(*gens):
                for g in gens:
                    yield from g

            def drive(gb, nb, ga, na, i0=0):
                done_a = 0
                for i in range(nb):
                    if next(gb, StopIteration) is StopIteration:
                        break
                    if ga is not None and i >= i0:
                        want = (i - i0 + 1) * na // max(1, nb - i0)
                        while done_a < want:
                            if next(ga, StopIteration) is StopIteration:
                                ga = None
                                break
                            done_a += 1
                for _ in gb:
                    pass
                if ga is not None:
                    for _ in ga:
                        pass

            def n_steps(qc):
                return (qc + 1) * 2 + 2

            # A(0) standalone; then stage j = B(j) [+proj(j)] with fills.
            # wk/wv/wp loads are issued only once chunk-0 Q work is underway
            # (gate: DVE op after the first q eviction) so the wq + x chunk 0
            # transfers get the full DMA bandwidth.
            ga0 = gen_A(0)
            for _ in range(3):
                next(ga0)
            # WAW gates: junk writes into the load destinations force the
            # wk/wv DMAs to wait until chunk-0 Q work is underway, leaving
            # the full DMA bandwidth to wq + x chunk 0 until then.
            q8 = qts[0][:, 0:1, 0:8]
            for wsb in (wk_sb, wv_sb):
                nc.vector.tensor_copy(wsb[:, 0:1, 0:8], q8)
                nc.vector.tensor_copy(wsb[:, 4:5, 0:8], q8)
            nc.sync.dma_start(out=wk_sb[:, 0:4, :], in_=wk_r[:, 0:4, :])
            nc.scalar.dma_start(out=wk_sb[:, 4:8, :], in_=wk_r[:, 4:8, :])
            nc.sync.dma_start(out=wv_sb[:, 0:4, :], in_=wv_r[:, 0:4, :])
            nc.scalar.dma_start(out=wv_sb[:, 4:8, :], in_=wv_r[:, 4:8, :])
            v4 = v_sb[:, :, :, :].rearrange("p t (h2 two) c -> p t h2 two c", two=2)
            nc.gpsimd.memset(v4[:, :, :, 0, D:P], 1.0)
            nc.gpsimd.memset(v4[:, :, :, 1, 0:D], 1.0)
            for _ in range(6):
                next(ga0)
            nc.vector.tensor_copy(wp_sb[:, 0:1, 0:8], kT_sb[:, 0:1, 0:8])
            nc.gpsimd.dma_start(out=wp_sb, in_=wp_r)
            for _ in ga0:
                pass
            def merge(g1, n1, g2, n2):
                # round-robin weighted by counts, g1-first
                a, b = n1, n2
                while a or b:
                    if a * n2 >= b * n1 and a:
                        yield next(g1); a -= 1
                    elif b:
                        yield next(g2); b -= 1

            NA = 24
            proj12 = chain(gen_proj(1), gen_proj(2))
            for j in range(NTCH):
                gb = chain(*[gen_Bpair(j, p) for p in range(NPAIR)])
                nb = NPAIR * n_steps(j)
                if j == 0:
                    ga, na = gen_A(1), NA
                elif j == 1:
                    ga = merge(gen_A(2), NA, gen_proj(0), 8)
                    na = NA + 8
                elif j == 2:
                    ga, na = gen_A(3), NA
                else:
                    # proj(1) + proj(2) spread through the ACT-paced last
                    # stage; 3 groups held back to cover the final norm.
                    ga, na = (x for k, x in zip(range(13), proj12)), 13
                drive(gb, nb, ga, na, 0)
            for _ in proj12:
                pass
            for _ in gen_proj(3, evict_on_act=True):
                pass
    nc.finalize()
    return nc


_CACHE = {}


def _get_nc():
    if "nc" not in _CACHE:
        _CACHE["nc"] = build_nc()
    return _CACHE["nc"]


def make_in_maps(x, Wq, bq, Wk, bk, Wv, bv, Wp, bp):
    f = np.float32
    BF = ml_dtypes.bfloat16
    x = np.asarray(x, f)
    in_maps = []
    for core in range(8):
        b, g = core // 2, core % 2
        sl = slice(g * GC, (g + 1) * GC)
        xT_ = x[b].T  # [C, T]
        xh = xT_.reshape(NCT, P, NTCH, TCH).transpose(2, 1, 0, 3)
        wq_ = np.asarray(Wq, f)[:, sl].reshape(NCT, P, GC).transpose(1, 0, 2)
        wk_ = np.asarray(Wk, f)[:, sl].reshape(NCT, P, GC).transpose(1, 0, 2)
        wv_ = np.asarray(Wv, f)[:, sl].reshape(NCT, P, GC).transpose(1, 0, 2)
        wp_ = np.asarray(Wp, f)[sl, :].reshape(NPAIR, P, C).transpose(1, 0, 2)
        in_maps.append(
            {
                "xh": np.ascontiguousarray(xh.astype(BF)),
                "wq": np.ascontiguousarray(wq_.astype(BF)),
                "wk": np.ascontiguousarray(wk_.astype(BF)),
                "wv": np.ascontiguousarray(wv_.astype(BF)),
                "wp": np.ascontiguousarray(wp_.astype(BF)),
                "bq": np.ascontiguousarray(np.asarray(bq, f)[sl].reshape(NPAIR, P).T),
                "bk": np.ascontiguousarray(np.asarray(bk, f)[sl].reshape(NPAIR, P).T),
                "bv": np.ascontiguousarray(np.asarray(bv, f)[sl]),
            }
        )
    return in_maps


def run(in_maps, **kwargs):
    from concourse.bass_utils import run_bass_kernel_spmd

    return run_bass_kernel_spmd(
        _get_nc(), in_maps, core_ids=list(range(8)), **kwargs
    )


def kernel(x, Wq, bq, Wk, bk, Wv, bv, Wp, bp):
    in_maps = make_in_maps(x, Wq, bq, Wk, bk, Wv, bv, Wp, bp)
    res = run(in_maps)
    ys = [r["y"] for r in res.results]
    bp_f = np.asarray(bp, np.float32)
    out = np.stack([ys[2 * b] + ys[2 * b + 1] + bp_f[None, :] for b in range(B)])
    return out.astype(np.float32)


# revision 3
# speedup vs baseline: 1.0174x; 1.0143x over previous
"""Causal self-attention kernel for 8 trn2 NeuronCores — v3.

Sharding: core c = (b, g): b = c // 2 (batch), g = c % 2 (head-group of 8
heads, Megatron column split of Wq/Wk/Wv + row split of Wp). Host sums the
two partial Y per batch and adds bp.

Design (all matmul operands bf16, PSUM f32):
  - One streamed schedule: projection chunk j+1 and the hoisted first pair of
    the last attention chunk are interleaved into attention chunk j at
    matmul-group granularity, keeping the PE busy while ACT works through
    the softmax exps.
  - QK per k-tile: two concurrent 64x128 row-tile matmuls (head-even
    partitions 0:63 / head-odd 64:127) into separate PSUM banks.
  - V tiles augmented per head: even heads [V | 1], odd heads [1 | V]; the
    PV matmul leaves O on one partition half and the softmax colsum
    (replicated x64) on the other. Normalization: 2 cross-half copies
    assemble both colsum blocks into one full tile, one
    reciprocal_approx_fast, 2 DVE mults write A^T directly (no DMA).
  - Diagonal-block trimming: QK/exp/PV restricted to live columns
    [128*tloc:512]; dead pt columns zeroed by gpsimd memset; causal
    affine_select only on the 128-column boundary window.
  - Startup weight DMAs split across engine queues so the first projection
    matmul issues ~10us after kernel start.
"""

import numpy as np
import ml_dtypes

import concourse.bacc as bacc
import concourse.bass as bass
import concourse.mybir as mybir
import concourse.tile as tile

F32 = mybir.dt.float32
BF16 = mybir.dt.bfloat16
AF = mybir.ActivationFunctionType
ALU = mybir.AluOpType

B, T, C = 4, 2048, 1024
H, D = 16, 64
G = 2
GC = C // G  # 512
P = 128
NCT = C // P  # 8
TCH = 512
NTCH = T // TCH  # 4
NPAIR = GC // P  # 4
NKT = T // P  # 16
HPC = H // G  # 8
LAG = 2


def bcast_ap(h, parts, free):
    ap = h[:]
    return bass.AP(tensor=ap.tensor, offset=ap.offset, ap=[[0, parts], [1, free]])


def build_nc():
    nc = bacc.Bacc("TRN2", target_bir_lowering=False)

    # all inputs host-prearranged into on-chip layout: DMA lines are long
    # and contiguous per partition (8KB), not 1KB gather packets.
    xh = nc.dram_tensor("xh", [NTCH, P, NCT, TCH], BF16, kind="ExternalInput")
    wq = nc.dram_tensor("wq", [P, NCT, GC], BF16, kind="ExternalInput")
    wk = nc.dram_tensor("wk", [P, NCT, GC], BF16, kind="ExternalInput")
    wv = nc.dram_tensor("wv", [P, NCT, GC], BF16, kind="ExternalInput")
    wp = nc.dram_tensor("wp", [P, NPAIR, C], BF16, kind="ExternalInput")
    bqd = nc.dram_tensor("bq", [P, NPAIR], F32, kind="ExternalInput")
    bkd = nc.dram_tensor("bk", [P, NPAIR], F32, kind="ExternalInput")
    bvd = nc.dram_tensor("bv", [GC], F32, kind="ExternalInput")
    y = nc.dram_tensor("y", [T, C], F32, kind="ExternalOutput")

    wq_r, wk_r, wv_r, wp_r = wq[:, :, :], wk[:, :, :], wv[:, :, :], wp[:, :, :]
    y_r = y[:, :].rearrange("(n p) c -> n p c", p=P)

    with tile.TileContext(nc) as tc:
        with (
            tc.tile_pool(name="persist", bufs=1) as persist,
            tc.tile_pool(name="small", bufs=1) as small,
            tc.tile_pool(name="qpool", bufs=2) as qpool,
            tc.tile_pool(name="xpool", bufs=2) as xpool,
            tc.tile_pool(name="ptpool", bufs=4) as ptpool,
            tc.tile_pool(name="hoistpool", bufs=8) as hoistpool,
            tc.tile_pool(name="atpool", bufs=17) as atpool,
            tc.tile_pool(name="cspool", bufs=2) as cspool,
            tc.tile_pool(name="rcpool", bufs=2) as rcpool,
            tc.tile_pool(name="ypool", bufs=3) as ypool,
            tc.tile_pool(name="prpsum", bufs=2, space="PSUM") as prpsum,
            tc.tile_pool(name="stpsum", bufs=2, space="PSUM") as stpsum,
            tc.tile_pool(name="opsum", bufs=2, space="PSUM") as opsum,
        ):
            kT_sb = persist.tile([P, NPAIR, T], BF16)
            v_sb = persist.tile([P, NKT, HPC, P], BF16)
            wp_sb = persist.tile([P, NPAIR, C], BF16)
            wq_sb = persist.tile([P, NCT, GC], BF16)
            wk_sb = persist.tile([P, NCT, GC], BF16)
            wv_sb = persist.tile([P, NCT, GC], BF16)
            # spread the startup loads over engine queues; wq + x chunk 0
            # gate the first matmul, so they go first on separate queues.
            nc.sync.dma_start(out=wq_sb[:, 0:4, :], in_=wq_r[:, 0:4, :])
            nc.scalar.dma_start(out=wq_sb[:, 4:8, :], in_=wq_r[:, 4:8, :])
            xt0 = xpool.tile([P, NCT, TCH], BF16, name="xt")
            nc.sync.dma_start(out=xt0[:, 0:4, :], in_=xh[0, :, 0:4, :])
            nc.scalar.dma_start(out=xt0[:, 4:6, :], in_=xh[0, :, 4:6, :])
            nc.gpsimd.dma_start(out=xt0[:, 6:8, :], in_=xh[0, :, 6:8, :])
            # PE warmup during the input load: junk matmuls keep the HAM
            # clock gate at full rate so the first real matmuls run at 2.4GHz
            wtile = small.tile([P, TCH], BF16)
            nc.gpsimd.memset(wtile, 1.0)
            warm = opsum.tile([P, TCH], F32, name="ops")
            for wi in range(40):
                nc.tensor.matmul(
                    warm, lhsT=wtile[:, 0:P], rhs=wtile,
                    start=(wi == 0), stop=(wi == 39), skip_group_check=True,
                )

            bq_sb = small.tile([P, NPAIR], F32)
            nc.scalar.dma_start(out=bq_sb, in_=bqd[:, :])
            bk_sb = small.tile([P, NPAIR], F32)
            nc.scalar.dma_start(out=bk_sb, in_=bkd[:, :])

            # gpsimd queue: bv broadcast now; x chunk 0 next (inside gen_A(0));
            # ones-block memsets and wp (first needed ~25us / ~60us in) after.
            bv_sb = small.tile([P, GC], F32)
            nc.gpsimd.dma_start(out=bv_sb, in_=bcast_ap(bvd, P, GC))

            qts = {}
            aTs = {}
            pre_pts = {}

            def gen_A(tch):
                """QKV projections for t-chunk tch; yields after each MM group."""
                tsl = slice(tch * TCH, (tch + 1) * TCH)  # kT column range
                if tch == 0:
                    xt = xt0
                else:
                    xt = xpool.tile([P, NCT, TCH], BF16, name="xt")
                    nc.sync.dma_start(out=xt, in_=xh[tch, :, :, :])
                qT = qpool.tile([P, NPAIR, TCH], BF16, name="qT")
                qts[tch] = qT
                for pair in range(NPAIR):
                    psl = slice(pair * P, (pair + 1) * P)
                    q_ps = prpsum.tile([P, TCH], F32, name="pr")
                    for ct in range(NCT):
                        nc.tensor.matmul(
                            q_ps, lhsT=wq_sb[:, ct, psl], rhs=xt[:, ct, :],
                            start=(ct == 0), stop=(ct == NCT - 1),
                            skip_group_check=True,
                        )
                        if ct == 3:
                            yield
                    nc.vector.tensor_scalar_add(
                        qT[:, pair, :], q_ps, bq_sb[:, pair : pair + 1]
                    )
                    yield
                for pair in range(NPAIR):
                    psl = slice(pair * P, (pair + 1) * P)
                    k_ps = prpsum.tile([P, TCH], F32, name="pr")
                    for ct in range(NCT):
                        nc.tensor.matmul(
                            k_ps, lhsT=wk_sb[:, ct, psl], rhs=xt[:, ct, :],
                            start=(ct == 0), stop=(ct == NCT - 1),
                            skip_group_check=True,
                        )
                        if ct == 3:
                            yield
                    nc.vector.tensor_scalar_add(
                        kT_sb[:, pair, tsl], k_ps, bk_sb[:, pair : pair + 1]
                    )
                    yield
                for tloc in range(4):
                    tt = tch * 4 + tloc
                    v_ps = prpsum.tile([P, GC], F32, name="pr")
                    for ct in range(NCT):
                        nc.tensor.matmul(
                            v_ps, lhsT=xt[:, ct, tloc * P : (tloc + 1) * P],
                            rhs=wv_sb[:, ct, :],
                            start=(ct == 0), stop=(ct == NCT - 1),
                            skip_group_check=True,
                        )
                        if ct == 3:
                            yield
                    vp4 = v_ps.rearrange("p (h2 two d) -> p h2 two d", two=2, d=D)
                    bv4 = bv_sb.rearrange("p (h2 two d) -> p h2 two d", two=2, d=D)
                    vd4 = v_sb[:, tt, :, :].rearrange(
                        "p (h2 two) c -> p h2 two c", two=2
                    )
                    nc.vector.tensor_tensor(
                        vd4[:, :, 0, 0:D], vp4[:, :, 0, :], bv4[:, :, 0, :], ALU.add
                    )
                    nc.vector.tensor_tensor(
                        vd4[:, :, 1, D:P], vp4[:, :, 1, :], bv4[:, :, 1, :], ALU.add
                    )
                    yield

            def gen_QKhoist(qc, pair, t_list):
                """QK + exp only (non-diagonal tiles), pt kept in hoistpool."""
                qT = qts[qc]
                for t in t_list:
                    ksl = slice(t * P, (t + 1) * P)
                    st = stpsum.tile([P, 2, TCH], F32, name="st")
                    nc.tensor.matmul(
                        st[:, 0, :], lhsT=kT_sb[0:D, pair, ksl],
                        rhs=qT[0:D, pair, :], start=True, stop=True,
                    )
                    nc.tensor.matmul(
                        st[:, 1, :], lhsT=kT_sb[D:P, pair, ksl],
                        rhs=qT[D:P, pair, :], start=True, stop=True,
                    )
                    pt = hoistpool.tile([P, 2, TCH], BF16, name="hpt")
                    pre_pts[(qc, pair, t)] = pt
                    nc.scalar.activation(
                        out=pt[:, :, :], in_=st[:, :, :], func=AF.Exp, scale=0.125,
                    )
                    yield

            def gen_Bpair(qc, pair, pre_n=0):
                """Attention for (q-chunk qc, head pair); yields per step."""
                nkt = (qc + 1) * 4
                qT = qts[qc]
                o_e = opsum.tile([P, TCH], F32, name="ops")
                o_o = opsum.tile([P, TCH], F32, name="ops")
                pts = {t: pre_pts[(qc, pair, t)] for t in range(pre_n)}
                c0s = {t: 0 for t in range(pre_n)}
                # 2 ktiles per iteration: QK,QK then PV,PV halves the
                # row-mode <-> full-mode transitions on the PE.
                for it in range(nkt // 2 + 1):
                    for t in (2 * it, 2 * it + 1):
                        if t >= nkt or t < pre_n:
                            continue
                        ksl = slice(t * P, (t + 1) * P)
                        c0 = P * (t - 4 * qc) if t >= 4 * qc else 0
                        c0s[t] = c0
                        csl = slice(c0, TCH)
                        st = stpsum.tile([P, 2, TCH], F32, name="st")
                        nc.tensor.matmul(
                            st[:, 0, csl], lhsT=kT_sb[0:D, pair, ksl],
                            rhs=qT[0:D, pair, csl], start=True, stop=True,
                        )
                        nc.tensor.matmul(
                            st[:, 1, csl], lhsT=kT_sb[D:P, pair, ksl],
                            rhs=qT[D:P, pair, csl], start=True, stop=True,
                        )
                        pt = ptpool.tile([P, 2, TCH], BF16, name="pt")
                        pts[t] = pt
                        if c0 > 0:
                            nc.gpsimd.memset(pt[:, :, 0:c0], 0.0)
                        nc.scalar.activation(
                            out=pt[:, :, csl], in_=st[:, :, csl],
                            func=AF.Exp, scale=0.125,
                        )
                        if t >= 4 * qc:
                            # boundary window [c0:c0+P): keep iff w >= p
                            nc.gpsimd.affine_select(
                                out=pt[:, :, c0 : c0 + P],
                                in_=pt[:, :, c0 : c0 + P],
                                pattern=[[0, 2], [1, P]],
                                compare_op=ALU.is_ge, fill=0.0,
                                base=0, channel_multiplier=-1,
                            )
                    for pv in (2 * it - LAG, 2 * it - LAG + 1):
                        if not (0 <= pv < nkt):
                            continue
                        pcsl = slice(c0s[pv], TCH)
                        nc.tensor.matmul(
                            o_e[:, pcsl], lhsT=v_sb[:, pv, 2 * pair, :],
                            rhs=pts[pv][:, 0, pcsl],
                            start=(pv == 0), stop=(pv == nkt - 1),
                            skip_group_check=True,
                        )
                        nc.tensor.matmul(
                            o_o[:, pcsl], lhsT=v_sb[:, pv, 2 * pair + 1, :],
                            rhs=pts[pv][:, 1, pcsl],
                            start=(pv == 0), stop=(pv == nkt - 1),
                            skip_group_check=True,
                        )
                        del pts[pv]
                    yield
                # normalization: colsums at o_e[D:P] and o_o[0:D]
                cs = cspool.tile([P, TCH], F32, name="cs")
                nc.vector.tensor_copy(cs[0:D, :], o_e[D:P, :])
                nc.vector.tensor_copy(cs[D:P, :], o_o[0:D, :])
                rc = rcpool.tile([P, TCH], F32, name="rc")
                nc.vector.reciprocal_approx_fast(out=rc, in_=cs)
                aT = atpool.tile([P, TCH], BF16, name="aT")
                nc.vector.tensor_tensor(aT[0:D, :], o_e[0:D, :], rc[0:D, :], ALU.mult)
                nc.vector.tensor_tensor(aT[D:P, :], o_o[D:P, :], rc[D:P, :], ALU.mult)
                aTs[(qc, pair)] = aT
                yield

            def gen_proj(qc, evict_on_act=False):
                for tloc in range(4):
                    trow = qc * 4 + tloc
                    lsl = slice(tloc * P, (tloc + 1) * P)
                    for cch in range(2):
                        csl = slice(cch * TCH, (cch + 1) * TCH)
                        # pr rotation: free of A-work in the last stage, and
                        # never entangled with the o_e/o_o rotation.
                        y_ps = prpsum.tile([P, TCH], F32, name="pr")
                        for pair in range(NPAIR):
                            nc.tensor.matmul(
                                y_ps, lhsT=aTs[(qc, pair)][:, lsl],
                                rhs=wp_sb[:, pair, csl],
                                start=(pair == 0), stop=(pair == NPAIR - 1),
                                skip_group_check=True,
                            )
                        y_sb = ypool.tile([P, TCH], F32, name="y_sb")
                        if evict_on_act:
                            nc.scalar.copy(y_sb, y_ps)
                            nc.scalar.dma_start(out=y_r[trow, :, csl], in_=y_sb)
                        else:
                            nc.vector.tensor_copy(y_sb, y_ps)
                            nc.sync.dma_start(out=y_r[trow, :, csl], in_=y_sb)
                        yield

            def chain(*gens):
                for g in gens:
                    yield from g

            def drive(gb, nb, ga, na, i0=0):
                done_a = 0
                for i in range(nb):
                    if next(gb, StopIteration) is StopIteration:
                        break
                    if ga is not None and i >= i0:
                        want = (i - i0 + 1) * na // max(1, nb - i0)
                        while done_a < want:
                            if next(ga, StopIteration) is StopIteration:
                                ga = None
                                break
                            done_a += 1
                for _ in gb:
                    pass
                if ga is not None:
                    for _ in ga:
                        pass

            def n_steps(qc):
                return (qc + 1) * 2 + 2

            # A(0) standalone; then stage j = B(j) [+proj(j)] with fills.
            # wk/wv/wp loads are issued only once chunk-0 Q work is underway
            # (gate: DVE op after the first q eviction) so the wq + x chunk 0
            # transfers get the full DMA bandwidth.
            ga0 = gen_A(0)
            for _ in range(3):
                next(ga0)
            # WAW gates: junk writes into the load destinations force the
            # wk/wv DMAs to wait until chunk-0 Q work is underway, leaving
            # the full DMA bandwidth to wq + x chunk 0 until then.
            q8 = qts[0][:, 0:1, 0:8]
            for wsb in (wk_sb, wv_sb):
                nc.vector.tensor_copy(wsb[:, 0:1, 0:8], q8)
                nc.vector.tensor_copy(wsb[:, 4:5, 0:8], q8)
            nc.sync.dma_start(out=wk_sb[:, 0:4, :], in_=wk_r[:, 0:4, :])
            nc.scalar.dma_start(out=wk_sb[:, 4:8, :], in_=wk_r[:, 4:8, :])
            nc.sync.dma_start(out=wv_sb[:, 0:4, :], in_=wv_r[:, 0:4, :])
            nc.scalar.dma_start(out=wv_sb[:, 4:8, :], in_=wv_r[:, 4:8, :])
            v4 = v_sb[:, :, :, :].rearrange("p t (h2 two) c -> p t h2 two c", two=2)
            nc.gpsimd.memset(v4[:, :, :, 0, D:P], 1.0)
            nc.gpsimd.memset(v4[:, :, :, 1, 0:D], 1.0)
            for _ in range(6):
                next(ga0)
            nc.vector.tensor_copy(wp_sb[:, 0:1, 0:8], kT_sb[:, 0:1, 0:8])
            nc.gpsimd.dma_start(out=wp_sb, in_=wp_r)
            for _ in ga0:
                pass
            def merge(g1, n1, g2, n2):
                # round-robin weighted by counts, g1-first
                a, b = n1, n2
                while a or b:
                    if a * n2 >= b * n1 and a:
                        yield next(g1); a -= 1
                    elif b:
                        yield next(g2); b -= 1

            NA = 24
            proj012 = chain(gen_proj(0), gen_proj(1), gen_proj(2))
            for j in range(NTCH):
                gb = chain(*[gen_Bpair(j, p) for p in range(NPAIR)])
                nb = NPAIR * n_steps(j)
                if j < 3:
                    ga, na = gen_A(j + 1), NA
                else:
                    # proj(0..2) spread through the ACT-paced last stage;
                    # 3 groups held back to cover the final norm.
                    ga, na = (x for k, x in zip(range(21), proj012)), 21
                drive(gb, nb, ga, na, 0)
            for _ in proj012:
                pass
            for _ in gen_proj(3, evict_on_act=True):
                pass
    nc.finalize()
    return nc


_CACHE = {}


def _get_nc():
    if "nc" not in _CACHE:
        _CACHE["nc"] = build_nc()
    return _CACHE["nc"]


def make_in_maps(x, Wq, bq, Wk, bk, Wv, bv, Wp, bp):
    f = np.float32
    BF = ml_dtypes.bfloat16
    x = np.asarray(x, f)
    in_maps = []
    for core in range(8):
        b, g = core // 2, core % 2
        sl = slice(g * GC, (g + 1) * GC)
        xT_ = x[b].T  # [C, T]
        xh = xT_.reshape(NCT, P, NTCH, TCH).transpose(2, 1, 0, 3)
        wq_ = np.asarray(Wq, f)[:, sl].reshape(NCT, P, GC).transpose(1, 0, 2)
        wk_ = np.asarray(Wk, f)[:, sl].reshape(NCT, P, GC).transpose(1, 0, 2)
        wv_ = np.asarray(Wv, f)[:, sl].reshape(NCT, P, GC).transpose(1, 0, 2)
        wp_ = np.asarray(Wp, f)[sl, :].reshape(NPAIR, P, C).transpose(1, 0, 2)
        in_maps.append(
            {
                "xh": np.ascontiguousarray(xh.astype(BF)),
                "wq": np.ascontiguousarray(wq_.astype(BF)),
                "wk": np.ascontiguousarray(wk_.astype(BF)),
                "wv": np.ascontiguousarray(wv_.astype(BF)),
                "wp": np.ascontiguousarray(wp_.astype(BF)),
                "bq": np.ascontiguousarray(np.asarray(bq, f)[sl].reshape(NPAIR, P).T),
                "bk": np.ascontiguousarray(np.asarray(bk, f)[sl].reshape(NPAIR, P).T),
                "bv": np.ascontiguousarray(np.asarray(bv, f)[sl]),
            }
        )
    return in_maps


def run(in_maps, **kwargs):
    from concourse.bass_utils import run_bass_kernel_spmd

    return run_bass_kernel_spmd(
        _get_nc(), in_maps, core_ids=list(range(8)), **kwargs
    )


def kernel(x, Wq, bq, Wk, bk, Wv, bv, Wp, bp):
    in_maps = make_in_maps(x, Wq, bq, Wk, bk, Wv, bv, Wp, bp)
    res = run(in_maps)
    ys = [r["y"] for r in res.results]
    bp_f = np.asarray(bp, np.float32)
    out = np.stack([ys[2 * b] + ys[2 * b + 1] + bp_f[None, :] for b in range(B)])
    return out.astype(np.float32)
